# revision 22
# baseline (speedup 1.0000x reference)
"""MultiHeadLatentAttention TRN2 kernel.

Sharding: 8 cores = 2 batches x 4 head-groups (4 heads of 128 dims each).
Each core computes, for its (batch, 4 heads):
    latT = Wdkv^T xT          [256, S]
    kT_h = Wuk_h^T latT       [hd, S]
    v_h  = latT^T Wuv_h       [S, hd]   (stationary latT, amortized LDW)
    qT_h = Wq_h^T xT          [hd, S]
    scoresT = k qT            [keys, q]   (transposed scores: no transposes)
    expT = exp(scale*scoresT) (causal: skip invalid blocks, tri-mask diagonal;
                               invalid strips memset to -30000 on DVE)
    den  = ones^T quadsum(expT)  [128, q]  (4 key-tiles per den matmul)
    ctxT = v^T expT / den     [hd, q]
    part = sum_h ctxT_h^T Wout_h  [S, dout]  (row-parallel partial, fp16 out)
Host sums the 4 partials per batch and adds b_out.

Out-proj is emitted in per-stt chunks interleaved between the next
q-block's attention heads so PE fills ACT-paced bubbles.
"""

import sys

_BASS_REPO = "/opt/trn_rl_repo"
if _BASS_REPO not in sys.path:
    sys.path.insert(0, _BASS_REPO)

import numpy as np

import concourse.bass as bass  # noqa: F401
import concourse.mybir as mybir
import concourse.tile as tile
from concourse import bacc, bass_utils

F32 = mybir.dt.float32
F16 = mybir.dt.float16

B = 2
S = 2048
DIN = 2048
DOUT = 2048
NH = 16
HD = 128
LAT = 256
NCORES = 8
HEADS_PER_CORE = 4
COLS_PER_CORE = HEADS_PER_CORE * HD  # 512

KC = DIN // 128  # 16 contraction chunks over d_in
NB = S // 512    # 4 blocks of 512 over S
NT = S // 128    # 16 tiles of 128 over S
SCALE = 1.0 / float(np.sqrt(HD))

_CACHE = {}


def _build():
    nc = bacc.Bacc("TRN2", target_bir_lowering=False, debug=False,
                   num_devices=NCORES)

    xt_d = nc.dram_tensor("xt", [DIN, S], F16, kind="ExternalInput")
    wq_d = nc.dram_tensor("wq", [DIN, COLS_PER_CORE], F16, kind="ExternalInput")
    wdkv_d = nc.dram_tensor("wdkv", [DIN, LAT], F16, kind="ExternalInput")
    wuk_d = nc.dram_tensor("wuk", [LAT, COLS_PER_CORE], F16, kind="ExternalInput")
    wuv_d = nc.dram_tensor("wuv", [LAT, COLS_PER_CORE], F16, kind="ExternalInput")
    wout_d = nc.dram_tensor("wout", [COLS_PER_CORE, DOUT], F16, kind="ExternalInput")
    mask_d = nc.dram_tensor("mask", [128, 128], F16, kind="ExternalInput")
    out_d = nc.dram_tensor("out", [S, DOUT], F16, kind="ExternalOutput")

    Exp = mybir.ActivationFunctionType.Exp

    with tile.TileContext(nc) as tc:
        with (
            tc.tile_pool(name="consts", bufs=1) as cpool,
            tc.tile_pool(name="wts", bufs=1) as wpool,
            tc.tile_pool(name="acts", bufs=1) as apool,
            tc.tile_pool(name="temps", bufs=1) as tpool,
        ):
            # ---- constants ----
            ones_t = cpool.tile([128, 512], F16, name="ones_t", tag="ones_t")
            nc.vector.memset(ones_t[:], 1.0)
            neg_t = cpool.tile([128, 128], F16, name="neg_t", tag="neg_t")
            nc.vector.memset(neg_t[:], -30000.0)
            mask_t = cpool.tile([128, 128], F16, name="mask_t", tag="mask_t")
            nc.scalar.dma_start(mask_t[:], mask_d.ap())

            # ---- weights / inputs ----
            # xt streams on both rings (feeds latT then qT); wdkv rides the
            # sync ring interleaved; wuk/wuv early on scalar; wq after xt;
            # wout late on scalar.
            # wuk/wuv are tiny and needed right after latT -- load them
            # FIRST on the scalar ring so kT/v never stall on them.
            wuk = []
            wuv = []
            for m in range(LAT // 128):
                t = wpool.tile([128, COLS_PER_CORE], F16, name=f"wuk{m}", tag=f"wuk{m}")
                nc.scalar.dma_start(t[:], wuk_d.ap()[128 * m:128 * (m + 1), :])
                wuk.append(t)
                t = wpool.tile([128, COLS_PER_CORE], F16, name=f"wuv{m}", tag=f"wuv{m}")
                nc.scalar.dma_start(t[:], wuv_d.ap()[128 * m:128 * (m + 1), :])
                wuv.append(t)
            xtA = []
            xtB = []
            wdkv = []
            for k in range(KC):
                ta = wpool.tile([128, S // 2], F16, name=f"xtA{k}", tag=f"xtA{k}")
                nc.sync.dma_start(ta[:], xt_d.ap()[128 * k:128 * (k + 1), 0:S // 2])
                tb = wpool.tile([128, S // 2], F16, name=f"xtB{k}", tag=f"xtB{k}")
                nc.scalar.dma_start(tb[:], xt_d.ap()[128 * k:128 * (k + 1), S // 2:S])
                xtA.append(ta)
                xtB.append(tb)
                t = wpool.tile([128, LAT], F16, name=f"wdkv{k}", tag=f"wdkv{k}")
                eng = nc.sync if k % 2 == 0 else nc.scalar
                eng.dma_start(t[:], wdkv_d.ap()[128 * k:128 * (k + 1), :])
                wdkv.append(t)

            def xt_sb(k, sb):
                t = xtA[k] if sb < 2 else xtB[k]
                return t[:, 512 * (sb % 2):512 * (sb % 2 + 1)]
            wq = []
            for k in range(KC):
                t = wpool.tile([128, COLS_PER_CORE], F16, name=f"wq{k}", tag=f"wq{k}")
                eng = nc.scalar if k % 2 == 0 else nc.sync
                eng.dma_start(t[:], wq_d.ap()[128 * k:128 * (k + 1), :])
                wq.append(t)
            wout = []
            for h in range(HEADS_PER_CORE):
                t = wpool.tile([128, DOUT], F16, name=f"wout{h}", tag=f"wout{h}")
                nc.sync.dma_start(t[:], wout_d.ap()[128 * h:128 * (h + 1), :])
                wout.append(t)

            # ---- persistent activations ----
            latT = [apool.tile([128, S], F16, name=f"latT{m}", tag=f"latT{m}")
                    for m in range(LAT // 128)]
            qT = [apool.tile([128, S], F16, name=f"qT{h}", tag=f"qT{h}")
                  for h in range(HEADS_PER_CORE)]
            kT = [apool.tile([128, S], F16, name=f"kT{h}", tag=f"kT{h}")
                  for h in range(HEADS_PER_CORE)]
            # vt_all column layout: [stt, h, d] -> col (4*stt + h)*128 + d
            vt_all = apool.tile([128, NT * 512], F16, name="vt_all", tag="vt_all")
            ctxT = [apool.tile([128, S], F16, name=f"ctxT{h}", tag=f"ctxT{h}")
                    for h in range(HEADS_PER_CORE)]

            # ================= phase 1: projections =================
            with tc.tile_pool(name="pproj", bufs=8, space="PSUM") as pproj:
                # HAM warm-keeper: stream dummy matmuls across the DMA ramp
                # so the PE is warm when the first real matmul issues.
                warm = pproj.tile([128, 512], F32, name="warm", tag="pp")
                for _ in range(72):
                    nc.tensor.matmul(warm[:, 0:128], ones_t[:, 0:128], ones_t[:, 0:128],
                                     start=True, stop=True)

                def kmajor(groups, lhs_of, rhs_of, nk, out_of, copy_eng,
                           filler=0):
                    """Accumulate len(groups) psum banks over nk chunks,
                    chunk-major so compute starts on the first DMA. filler>0
                    emits dummy ldweights between chunk iterations: they keep
                    the HAM clock-gate warm through DMA-paced stretches and
                    are harmless (every matmul reloads its own stationary)."""
                    pls = [pproj.tile([128, 512], F32, name=f"pp{i}", tag="pp")
                           for i in range(len(groups))]
                    for k in range(nk):
                        for i, g in enumerate(groups):
                            nc.tensor.matmul(pls[i][:], lhs_of(k, g), rhs_of(k, g),
                                             start=(k == 0), stop=(k == nk - 1))
                        if k < nk - 1:
                            for _ in range(filler):
                                nc.tensor.ldweights(ones_t[:, 0:128])
                    for i, g in enumerate(groups):
                        eng = copy_eng(i)
                        if eng == "scalar":
                            nc.scalar.copy(out_of(g), pls[i][:])
                        else:
                            nc.vector.tensor_copy(out_of(g), pls[i][:])

                # latT = Wdkv^T xT   (8 groups: 2 m x 4 sb)
                kmajor(
                    [(m, sb) for m in range(2) for sb in range(NB)],
                    lambda k, g: wdkv[k][:, 128 * g[0]:128 * (g[0] + 1)],
                    lambda k, g: xt_sb(k, g[1]),
                    KC,
                    lambda g: latT[g[0]][:, 512 * g[1]:512 * (g[1] + 1)],
                    lambda i: "scalar", filler=6)

                # kT_h = Wuk_h^T latT  (two rounds of 2h x 4sb, 2 chunks)
                for h0 in (0, 2):
                    kmajor(
                        [(h0 + dh, sb) for dh in range(2) for sb in range(NB)],
                        lambda k, g: wuk[k][:, 128 * g[0]:128 * (g[0] + 1)],
                        lambda k, g: latT[k][:, 512 * g[1]:512 * (g[1] + 1)],
                        2,
                        lambda g: kT[g[0]][:, 512 * g[1]:512 * (g[1] + 1)],
                        lambda i: "vector" if i % 2 else "scalar")

                # v_h = latT^T Wuv_h: stationary latT s-tile reused across
                # the 4 heads (LDW amortized). Each m-chunk goes to its own
                # bank as independent start/stop groups (interleaved groups
                # on one bank would clear each other's has_written bits);
                # the two banks are summed on the way to SBUF.
                for stt in range(NT):
                    pv0 = pproj.tile([128, 512], F32, name="pv0", tag="pp")
                    pv1 = pproj.tile([128, 512], F32, name="pv1", tag="pp")
                    for m, pv in ((0, pv0), (1, pv1)):
                        for h in range(HEADS_PER_CORE):
                            nc.tensor.matmul(
                                pv[:, 128 * h:128 * (h + 1)],
                                latT[m][:, 128 * stt:128 * (stt + 1)],
                                wuv[m][:, 128 * h:128 * (h + 1)],
                                start=True, stop=True,
                                skip_group_check=True)
                    vtmp = tpool.tile([128, 512], F16, name="vtmp",
                                      tag="vtmp", bufs=2)
                    nc.scalar.copy(vtmp[:], pv1[:])
                    nc.vector.tensor_add(
                        vt_all[:, 512 * stt:512 * (stt + 1)], pv0[:], vtmp[:])

                # qT_h = Wq_h^T xT (last: its tail is the phase barrier;
                # copies split across both engines to drain fast)
                for h0 in (0, 2):
                    kmajor(
                        [(h0 + dh, sb) for dh in range(2) for sb in range(NB)],
                        lambda k, g: wq[k][:, 128 * g[0]:128 * (g[0] + 1)],
                        lambda k, g: xt_sb(k, g[1]),
                        KC,
                        lambda g: qT[g[0]][:, 512 * g[1]:512 * (g[1] + 1)],
                        lambda i: "vector" if i % 2 else "scalar")

            # ========= phase 2: attention + interleaved out-proj =========
            with (
                tc.tile_pool(name="psc", bufs=2, space="PSUM") as psc,
                tc.tile_pool(name="pctx", bufs=2, space="PSUM") as pctx,
                tc.tile_pool(name="pden", bufs=2, space="PSUM") as pden,
            ):
                def vslice(kt, h):
                    c0 = (4 * kt + h) * 128
                    return vt_all[:, c0:c0 + 128]

                def attention(qb, h):
                    ps_ctx = pctx.tile([128, 512], F32, name="ps_ctx", tag="ctx")
                    ps_den = pden.tile([128, 512], F32, name="ps_den", tag="den")
                    nkt = 4 * qb + 4
                    npair = nkt // 2
                    state = {}

                    def part1(p):
                        # scores + exp + mask for pair p
                        kt0 = 2 * p
                        pair = (kt0, kt0 + 1)
                        djA = pair[0] - 4 * qb
                        col0 = 128 * djA if djA >= 0 else 0
                        qhi = 512 * (qb + 1)
                        ps_sc = psc.tile([128, 1024], F32, name="ps_sc",
                                         tag="sc")
                        ex = tpool.tile([128, 1024], F16, name="ex", tag="ex",
                                        bufs=3)
                        # invalid strip of half 1 (valid starts 128 later
                        # than half 0): memset -huge on DVE so the wide exp
                        # lands exact zeros there (was a PE fill matmul).
                        dj1 = pair[1] - 4 * qb
                        if dj1 > 0:
                            nc.vector.memset(
                                ps_sc[:, 512 + col0:512 + 128 * dj1],
                                -30000.0)
                        for half, kt in enumerate(pair):
                            dj = kt - 4 * qb
                            c = 128 * dj if dj >= 0 else 0
                            nc.tensor.matmul(
                                ps_sc[:, 512 * half + c:512 * (half + 1)],
                                kT[h][:, 128 * kt:128 * (kt + 1)],
                                qT[h][:, 512 * qb + c:qhi],
                                start=True, stop=True,
                                skip_group_check=True)
                        # one wide exp for the pair (psum -> sbuf fp16)
                        nc.scalar.activation(ex[:, col0:1024],
                                             ps_sc[:, col0:1024], Exp,
                                             scale=SCALE)
                        for half, kt in enumerate(pair):
                            dj = kt - 4 * qb
                            if dj >= 0:
                                c = 128 * dj
                                sl = ex[:, 512 * half + c:512 * half + c + 128]
                                nc.vector.tensor_mul(sl, sl, mask_t[:])
                        state[p] = (ex, col0)

                    def part2(p):
                        # ctx + den matmuls for pair p
                        kt0 = 2 * p
                        pair = (kt0, kt0 + 1)
                        ex, col0 = state.pop(p)
                        for half, kt in enumerate(pair):
                            nc.tensor.matmul(
                                ps_ctx[:, col0:512],
                                vslice(kt, h),
                                ex[:, 512 * half + col0:512 * (half + 1)],
                                start=(kt0 == 0 and half == 0),
                                stop=(kt == nkt - 1))
                        # pair-sum on DVE; den matmul once per quad
                        exs = tpool.tile([128, 512], F16, name="exs",
                                         tag="exs", bufs=3)
                        if col0 > 0:
                            nc.vector.memset(exs[:, 0:col0], 0.0)
                        nc.vector.tensor_add(exs[:, col0:512],
                                             ex[:, col0:512],
                                             ex[:, 512 + col0:1024])
                        if p % 2 == 0:
                            state['exs'] = exs
                        else:
                            quad = tpool.tile([128, 512], F16, name="quad",
                                              tag="quad", bufs=2)
                            nc.vector.tensor_add(quad[:], state.pop('exs'),
                                                 exs[:])
                            nc.tensor.matmul(
                                ps_den[:], ones_t[:, 0:128], quad[:],
                                start=(p == 1), stop=(p == npair - 1))

                    # one-deep software pipeline: pair p+1's scores issue
                    # between pair p's scores and its ctx/den matmuls, so
                    # the PE fills the exp/mask latency.
                    part1(0)
                    for p in range(1, npair):
                        part1(p)
                        part2(p - 1)
                    part2(npair - 1)
                    rden = tpool.tile([128, 512], F32, name="rden", tag="rden",
                                      bufs=2)
                    nc.vector.reciprocal_approx_fast(rden[:], ps_den[:])
                    nc.vector.tensor_mul(ctxT[h][:, 512 * qb:512 * (qb + 1)],
                                         ps_ctx[:], rden[:])

                def outproj_chunk(stt, tail):
                    for ob in range(NB):
                        po = pden.tile([128, 512], F32, name="po", tag="den")
                        for h in range(HEADS_PER_CORE):
                            nc.tensor.matmul(
                                po[:],
                                ctxT[h][:, 128 * stt:128 * (stt + 1)],
                                wout[h][:, 512 * ob:512 * (ob + 1)],
                                start=(h == 0), stop=(h == HEADS_PER_CORE - 1))
                        osb = tpool.tile([128, 512], F16, name="osb",
                                         tag="osb", bufs=3)
                        if tail and ob % 2 == 0:
                            nc.scalar.copy(osb[:], po[:])
                            st_eng = nc.scalar
                        else:
                            nc.vector.tensor_copy(osb[:], po[:])
                            st_eng = nc.sync if ob % 2 == 0 else nc.scalar
                        st_eng.dma_start(
                            out_d.ap()[128 * stt:128 * (stt + 1),
                                       512 * ob:512 * (ob + 1)],
                            osb[:])

                # out-proj chunks of q-block qb-1 interleave between the
                # attention heads of q-block qb (fills ACT-paced bubbles)
                # ldweights bridge across the pool-swap barrier (no PSUM
                # dependency, so they run while the projection pools drain)
                for _ in range(16):
                    nc.tensor.ldweights(ones_t[:, 0:128])
                for qb in range(NB):
                    for h in range(HEADS_PER_CORE):
                        attention(qb, h)
                        if qb <= 1:
                            for _ in range(8):
                                nc.tensor.ldweights(ones_t[:, 0:128])
                        if qb > 0:
                            outproj_chunk(4 * (qb - 1) + h, tail=False)
                for stt in range(12, 16):
                    outproj_chunk(stt, tail=True)

    nc.compile()
    return nc


def _get_nc():
    if "nc" not in _CACHE:
        _CACHE["nc"] = _build()
    return _CACHE["nc"]


def _make_in_maps(x, W_query, W_DKV, W_UK, W_UV, W_out):
    mask = np.triu(np.ones((128, 128), dtype=np.float16))
    wdkv16 = W_DKV.astype(np.float16)
    xT16 = [x[b].T.astype(np.float16) for b in range(B)]
    in_maps = []
    for c in range(NCORES):
        b = c // 4
        g = c % 4
        cols = slice(512 * g, 512 * (g + 1))
        in_maps.append({
            "xt": xT16[b],
            "wq": W_query[:, cols].astype(np.float16),
            "wdkv": wdkv16,
            "wuk": W_UK[:, cols].astype(np.float16),
            "wuv": W_UV[:, cols].astype(np.float16),
            "wout": W_out[cols, :].astype(np.float16),
            "mask": mask,
        })
    return in_maps


def run_on_device(x, W_query, W_DKV, W_UK, W_UV, W_out, **run_kwargs):
    nc = _get_nc()
    in_maps = _make_in_maps(x, W_query, W_DKV, W_UK, W_UV, W_out)
    return bass_utils.run_bass_kernel_spmd(
        nc, in_maps, core_ids=list(range(NCORES)), **run_kwargs)


def kernel(x, W_query, W_DKV, W_UK, W_UV, W_out, b_out):
    x = np.asarray(x, dtype=np.float32)
    W_query = np.asarray(W_query, dtype=np.float32)
    W_DKV = np.asarray(W_DKV, dtype=np.float32)
    W_UK = np.asarray(W_UK, dtype=np.float32)
    W_UV = np.asarray(W_UV, dtype=np.float32)
    W_out = np.asarray(W_out, dtype=np.float32)
    b_out = np.asarray(b_out, dtype=np.float32)

    res = None
    for attempt in range(3):
        try:
            res = run_on_device(x, W_query, W_DKV, W_UK, W_UV, W_out)
            break
        except Exception:
            if attempt == 2:
                raise
    out = np.empty((B, S, DOUT), dtype=np.float32)
    for b in range(B):
        acc = res.results[4 * b]["out"].astype(np.float32)
        for g in range(1, 4):
            acc += res.results[4 * b + g]["out"].astype(np.float32)
        out[b] = acc + b_out[None, :]
    return out


# revision 23
# speedup vs baseline: 1.0210x; 1.0210x over previous
"""MultiHeadLatentAttention TRN2 kernel.

Sharding: 8 cores = 2 batches x 4 head-groups (4 heads of 128 dims each).
Each core computes, for its (batch, 4 heads):
    latT = Wdkv^T xT          [256, S]
    kT_h = Wuk_h^T latT       [hd, S]
    v_h  = latT^T Wuv_h       [S, hd]   (stationary latT, amortized LDW)
    qT_h = Wq_h^T xT          [hd, S]
    scoresT = k qT            [keys, q]   (transposed scores: no transposes)
    expT = exp(scale*scoresT) (causal: skip invalid blocks, tri-mask diagonal;
                               invalid strips memset to -30000 on DVE)
    den  = ones^T quadsum(expT)  [128, q]  (4 key-tiles per den matmul)
    ctxT = v^T expT / den     [hd, q]
    part = sum_h ctxT_h^T Wout_h  [S, dout]  (row-parallel partial, fp16 out)
Host sums the 4 partials per batch and adds b_out.

Out-proj is emitted in per-stt chunks interleaved between the next
q-block's attention heads so PE fills ACT-paced bubbles.
"""

import sys

_BASS_REPO = "/opt/trn_rl_repo"
if _BASS_REPO not in sys.path:
    sys.path.insert(0, _BASS_REPO)

import numpy as np

import concourse.bass as bass  # noqa: F401
import concourse.mybir as mybir
import concourse.tile as tile
from concourse import bacc, bass_utils

F32 = mybir.dt.float32
F16 = mybir.dt.float16

B = 2
S = 2048
DIN = 2048
DOUT = 2048
NH = 16
HD = 128
LAT = 256
NCORES = 8
HEADS_PER_CORE = 4
COLS_PER_CORE = HEADS_PER_CORE * HD  # 512

KC = DIN // 128  # 16 contraction chunks over d_in
NB = S // 512    # 4 blocks of 512 over S
NT = S // 128    # 16 tiles of 128 over S
SCALE = 1.0 / float(np.sqrt(HD))

_CACHE = {}


def _build():
    nc = bacc.Bacc("TRN2", target_bir_lowering=False, debug=False,
                   num_devices=NCORES)

    xt_d = nc.dram_tensor("xt", [DIN, S], F16, kind="ExternalInput")
    wq_d = nc.dram_tensor("wq", [DIN, COLS_PER_CORE], F16, kind="ExternalInput")
    wdkv_d = nc.dram_tensor("wdkv", [DIN, LAT], F16, kind="ExternalInput")
    wuk_d = nc.dram_tensor("wuk", [LAT, COLS_PER_CORE], F16, kind="ExternalInput")
    wuv_d = nc.dram_tensor("wuv", [LAT, COLS_PER_CORE], F16, kind="ExternalInput")
    wout_d = nc.dram_tensor("wout", [COLS_PER_CORE, DOUT], F16, kind="ExternalInput")
    mask_d = nc.dram_tensor("mask", [128, 128], F16, kind="ExternalInput")
    out_d = nc.dram_tensor("out", [S, DOUT], F16, kind="ExternalOutput")

    Exp = mybir.ActivationFunctionType.Exp

    with tile.TileContext(nc) as tc:
        with (
            tc.tile_pool(name="consts", bufs=1) as cpool,
            tc.tile_pool(name="wts", bufs=1) as wpool,
            tc.tile_pool(name="acts", bufs=1) as apool,
            tc.tile_pool(name="temps", bufs=1) as tpool,
        ):
            # ---- constants ----
            ones_t = cpool.tile([128, 512], F16, name="ones_t", tag="ones_t")
            nc.vector.memset(ones_t[:], 1.0)
            neg_t = cpool.tile([128, 128], F16, name="neg_t", tag="neg_t")
            nc.vector.memset(neg_t[:], -30000.0)
            mask_t = cpool.tile([128, 128], F16, name="mask_t", tag="mask_t")
            nc.scalar.dma_start(mask_t[:], mask_d.ap())

            # ---- weights / inputs ----
            # xt streams on both rings (feeds latT then qT); wdkv rides the
            # sync ring interleaved; wuk/wuv early on scalar; wq after xt;
            # wout late on scalar.
            # wuk/wuv are tiny and needed right after latT -- load them
            # FIRST on the scalar ring so kT/v never stall on them.
            wuk = []
            wuv = []
            for m in range(LAT // 128):
                t = wpool.tile([128, COLS_PER_CORE], F16, name=f"wuk{m}", tag=f"wuk{m}")
                nc.scalar.dma_start(t[:], wuk_d.ap()[128 * m:128 * (m + 1), :])
                wuk.append(t)
                t = wpool.tile([128, COLS_PER_CORE], F16, name=f"wuv{m}", tag=f"wuv{m}")
                nc.scalar.dma_start(t[:], wuv_d.ap()[128 * m:128 * (m + 1), :])
                wuv.append(t)
            xtA = []
            xtB = []
            wdkv = []
            for k in range(KC):
                ta = wpool.tile([128, S // 2], F16, name=f"xtA{k}", tag=f"xtA{k}")
                nc.sync.dma_start(ta[:], xt_d.ap()[128 * k:128 * (k + 1), 0:S // 2])
                tb = wpool.tile([128, S // 2], F16, name=f"xtB{k}", tag=f"xtB{k}")
                nc.scalar.dma_start(tb[:], xt_d.ap()[128 * k:128 * (k + 1), S // 2:S])
                xtA.append(ta)
                xtB.append(tb)
                t = wpool.tile([128, LAT], F16, name=f"wdkv{k}", tag=f"wdkv{k}")
                eng = nc.sync if k % 2 == 0 else nc.scalar
                eng.dma_start(t[:], wdkv_d.ap()[128 * k:128 * (k + 1), :])
                wdkv.append(t)

            def xt_sb(k, sb):
                t = xtA[k] if sb < 2 else xtB[k]
                return t[:, 512 * (sb % 2):512 * (sb % 2 + 1)]
            wq = []
            for k in range(KC):
                t = wpool.tile([128, COLS_PER_CORE], F16, name=f"wq{k}", tag=f"wq{k}")
                eng = nc.scalar if k % 2 == 0 else nc.sync
                eng.dma_start(t[:], wq_d.ap()[128 * k:128 * (k + 1), :])
                wq.append(t)
            wout = []
            for h in range(HEADS_PER_CORE):
                t = wpool.tile([128, DOUT], F16, name=f"wout{h}", tag=f"wout{h}")
                nc.sync.dma_start(t[:], wout_d.ap()[128 * h:128 * (h + 1), :])
                wout.append(t)

            # ---- persistent activations ----
            latT = [apool.tile([128, S], F16, name=f"latT{m}", tag=f"latT{m}")
                    for m in range(LAT // 128)]
            qT = [apool.tile([128, S], F16, name=f"qT{h}", tag=f"qT{h}")
                  for h in range(HEADS_PER_CORE)]
            kT = [apool.tile([128, S], F16, name=f"kT{h}", tag=f"kT{h}")
                  for h in range(HEADS_PER_CORE)]
            # vt_all column layout: [stt, h, d] -> col (4*stt + h)*128 + d
            vt_all = apool.tile([128, NT * 512], F16, name="vt_all", tag="vt_all")
            ctxT = [apool.tile([128, S], F16, name=f"ctxT{h}", tag=f"ctxT{h}")
                    for h in range(HEADS_PER_CORE)]

            # ================= phase 1: projections =================
            with tc.tile_pool(name="pproj", bufs=8, space="PSUM") as pproj:
                # HAM warm-keeper: stream dummy matmuls across the DMA ramp
                # so the PE is warm when the first real matmul issues.
                warm = pproj.tile([128, 512], F32, name="warm", tag="pp")
                for _ in range(72):
                    nc.tensor.matmul(warm[:, 0:128], ones_t[:, 0:128], ones_t[:, 0:128],
                                     start=True, stop=True)

                def kmajor(groups, lhs_of, rhs_of, nk, out_of, copy_eng,
                           filler=0):
                    """Accumulate len(groups) psum banks over nk chunks,
                    chunk-major so compute starts on the first DMA. filler>0
                    emits dummy ldweights between chunk iterations: they keep
                    the HAM clock-gate warm through DMA-paced stretches and
                    are harmless (every matmul reloads its own stationary)."""
                    pls = [pproj.tile([128, 512], F32, name=f"pp{i}", tag="pp")
                           for i in range(len(groups))]
                    for k in range(nk):
                        for i, g in enumerate(groups):
                            nc.tensor.matmul(pls[i][:], lhs_of(k, g), rhs_of(k, g),
                                             start=(k == 0), stop=(k == nk - 1))
                        if k < nk - 1:
                            for _ in range(filler):
                                nc.tensor.ldweights(ones_t[:, 0:128])
                    for i, g in enumerate(groups):
                        eng = copy_eng(i)
                        if eng == "scalar":
                            nc.scalar.copy(out_of(g), pls[i][:])
                        else:
                            nc.vector.tensor_copy(out_of(g), pls[i][:])

                # latT = Wdkv^T xT   (8 groups: 2 m x 4 sb)
                kmajor(
                    [(m, sb) for m in range(2) for sb in range(NB)],
                    lambda k, g: wdkv[k][:, 128 * g[0]:128 * (g[0] + 1)],
                    lambda k, g: xt_sb(k, g[1]),
                    KC,
                    lambda g: latT[g[0]][:, 512 * g[1]:512 * (g[1] + 1)],
                    lambda i: "scalar", filler=6)

                # kT_h = Wuk_h^T latT  (two rounds of 2h x 4sb, 2 chunks)
                for h0 in (0, 2):
                    kmajor(
                        [(h0 + dh, sb) for dh in range(2) for sb in range(NB)],
                        lambda k, g: wuk[k][:, 128 * g[0]:128 * (g[0] + 1)],
                        lambda k, g: latT[k][:, 512 * g[1]:512 * (g[1] + 1)],
                        2,
                        lambda g: kT[g[0]][:, 512 * g[1]:512 * (g[1] + 1)],
                        lambda i: "vector" if i % 2 else "scalar")

                # v_h = latT^T Wuv_h: stationary latT s-tile reused across
                # the 4 heads (LDW amortized). Each m-chunk goes to its own
                # bank as independent start/stop groups (interleaved groups
                # on one bank would clear each other's has_written bits);
                # the two banks are summed on the way to SBUF.
                for stt in range(NT):
                    pv0 = pproj.tile([128, 512], F32, name="pv0", tag="pp")
                    pv1 = pproj.tile([128, 512], F32, name="pv1", tag="pp")
                    for m, pv in ((0, pv0), (1, pv1)):
                        for h in range(HEADS_PER_CORE):
                            nc.tensor.matmul(
                                pv[:, 128 * h:128 * (h + 1)],
                                latT[m][:, 128 * stt:128 * (stt + 1)],
                                wuv[m][:, 128 * h:128 * (h + 1)],
                                start=True, stop=True,
                                skip_group_check=True)
                    vtmp = tpool.tile([128, 512], F16, name="vtmp",
                                      tag="vtmp", bufs=2)
                    nc.scalar.copy(vtmp[:], pv1[:])
                    nc.vector.tensor_add(
                        vt_all[:, 512 * stt:512 * (stt + 1)], pv0[:], vtmp[:])

                # qT_h = Wq_h^T xT (last: its tail is the phase barrier;
                # copies split across both engines to drain fast)
                for h0 in (0, 2):
                    kmajor(
                        [(h0 + dh, sb) for dh in range(2) for sb in range(NB)],
                        lambda k, g: wq[k][:, 128 * g[0]:128 * (g[0] + 1)],
                        lambda k, g: xt_sb(k, g[1]),
                        KC,
                        lambda g: qT[g[0]][:, 512 * g[1]:512 * (g[1] + 1)],
                        lambda i: "vector" if i % 2 else "scalar")

            # ========= phase 2: attention + interleaved out-proj =========
            with (
                tc.tile_pool(name="psc", bufs=2, space="PSUM") as psc,
                tc.tile_pool(name="pctx", bufs=2, space="PSUM") as pctx,
                tc.tile_pool(name="pden", bufs=2, space="PSUM") as pden,
            ):
                def vslice(kt, h):
                    c0 = (4 * kt + h) * 128
                    return vt_all[:, c0:c0 + 128]

                def attention(qb, h):
                    ps_ctx = pctx.tile([128, 512], F32, name="ps_ctx", tag="ctx")
                    ps_den = pden.tile([128, 512], F32, name="ps_den", tag="den")
                    nkt = 4 * qb + 4
                    npair = nkt // 2
                    state = {}

                    def part1(p):
                        # scores + exp + mask for pair p
                        kt0 = 2 * p
                        pair = (kt0, kt0 + 1)
                        djA = pair[0] - 4 * qb
                        col0 = 128 * djA if djA >= 0 else 0
                        qhi = 512 * (qb + 1)
                        ps_sc = psc.tile([128, 1024], F32, name="ps_sc",
                                         tag="sc")
                        ex = tpool.tile([128, 1024], F16, name="ex", tag="ex",
                                        bufs=3)
                        # invalid strip of half 1 (valid starts 128 later
                        # than half 0): memset -huge on DVE so the wide exp
                        # lands exact zeros there (was a PE fill matmul).
                        dj1 = pair[1] - 4 * qb
                        if dj1 > 0:
                            nc.vector.memset(
                                ps_sc[:, 512 + col0:512 + 128 * dj1],
                                -30000.0)
                        for half, kt in enumerate(pair):
                            dj = kt - 4 * qb
                            c = 128 * dj if dj >= 0 else 0
                            nc.tensor.matmul(
                                ps_sc[:, 512 * half + c:512 * (half + 1)],
                                kT[h][:, 128 * kt:128 * (kt + 1)],
                                qT[h][:, 512 * qb + c:qhi],
                                start=True, stop=True,
                                skip_group_check=True)
                        # one wide exp for the pair (psum -> sbuf fp16)
                        nc.scalar.activation(ex[:, col0:1024],
                                             ps_sc[:, col0:1024], Exp,
                                             scale=SCALE)
                        for half, kt in enumerate(pair):
                            dj = kt - 4 * qb
                            if dj >= 0:
                                c = 128 * dj
                                sl = ex[:, 512 * half + c:512 * half + c + 128]
                                nc.vector.tensor_mul(sl, sl, mask_t[:])
                        state[p] = (ex, col0)

                    def part2(p):
                        # ctx + den matmuls for pair p
                        kt0 = 2 * p
                        pair = (kt0, kt0 + 1)
                        ex, col0 = state.pop(p)
                        for half, kt in enumerate(pair):
                            nc.tensor.matmul(
                                ps_ctx[:, col0:512],
                                vslice(kt, h),
                                ex[:, 512 * half + col0:512 * (half + 1)],
                                start=(kt0 == 0 and half == 0),
                                stop=(kt == nkt - 1))
                        # pair-sum on DVE; den matmul once per quad
                        exs = tpool.tile([128, 512], F16, name="exs",
                                         tag="exs", bufs=3)
                        if col0 > 0:
                            nc.vector.memset(exs[:, 0:col0], 0.0)
                        nc.vector.tensor_add(exs[:, col0:512],
                                             ex[:, col0:512],
                                             ex[:, 512 + col0:1024])
                        if p % 2 == 0:
                            state['exs'] = exs
                        else:
                            quad = tpool.tile([128, 512], F16, name="quad",
                                              tag="quad", bufs=3)
                            nc.vector.tensor_add(quad[:], state.pop('exs'),
                                                 exs[:])
                            nc.tensor.matmul(
                                ps_den[:], ones_t[:, 0:128], quad[:],
                                start=(p == 1), stop=(p == npair - 1))

                    # one-deep software pipeline: pair p+1's scores issue
                    # between pair p's scores and its ctx/den matmuls, so
                    # the PE fills the exp/mask latency.
                    part1(0)
                    for p in range(1, npair):
                        part1(p)
                        part2(p - 1)
                    part2(npair - 1)
                    rden = tpool.tile([128, 512], F32, name="rden", tag="rden",
                                      bufs=3)
                    nc.vector.reciprocal_approx_fast(rden[:], ps_den[:])
                    nc.vector.tensor_mul(ctxT[h][:, 512 * qb:512 * (qb + 1)],
                                         ps_ctx[:], rden[:])

                def outproj_chunk(stt, tail):
                    for ob in range(NB):
                        po = pden.tile([128, 512], F32, name="po", tag="den")
                        for h in range(HEADS_PER_CORE):
                            nc.tensor.matmul(
                                po[:],
                                ctxT[h][:, 128 * stt:128 * (stt + 1)],
                                wout[h][:, 512 * ob:512 * (ob + 1)],
                                start=(h == 0), stop=(h == HEADS_PER_CORE - 1))
                        osb = tpool.tile([128, 512], F16, name="osb",
                                         tag="osb", bufs=4)
                        if ob % 2 == 0:
                            nc.scalar.copy(osb[:], po[:])
                        else:
                            nc.vector.tensor_copy(osb[:], po[:])
                        st_eng = nc.sync if ob % 2 == 0 else nc.scalar
                        st_eng.dma_start(
                            out_d.ap()[128 * stt:128 * (stt + 1),
                                       512 * ob:512 * (ob + 1)],
                            osb[:])

                # out-proj chunks of q-block qb-1 interleave between the
                # attention heads of q-block qb (fills ACT-paced bubbles)
                # ldweights bridge across the pool-swap barrier (no PSUM
                # dependency, so they run while the projection pools drain)
                for _ in range(16):
                    nc.tensor.ldweights(ones_t[:, 0:128])
                for qb in range(NB):
                    for h in range(HEADS_PER_CORE):
                        attention(qb, h)
                        if qb <= 1:
                            for _ in range(8):
                                nc.tensor.ldweights(ones_t[:, 0:128])
                        if qb > 0:
                            outproj_chunk(4 * (qb - 1) + h, tail=False)
                for stt in range(12, 16):
                    outproj_chunk(stt, tail=True)

    nc.compile()
    return nc


def _get_nc():
    if "nc" not in _CACHE:
        _CACHE["nc"] = _build()
    return _CACHE["nc"]


def _make_in_maps(x, W_query, W_DKV, W_UK, W_UV, W_out):
    mask = np.triu(np.ones((128, 128), dtype=np.float16))
    wdkv16 = W_DKV.astype(np.float16)
    xT16 = [x[b].T.astype(np.float16) for b in range(B)]
    in_maps = []
    for c in range(NCORES):
        b = c // 4
        g = c % 4
        cols = slice(512 * g, 512 * (g + 1))
        in_maps.append({
            "xt": xT16[b],
            "wq": W_query[:, cols].astype(np.float16),
            "wdkv": wdkv16,
            "wuk": W_UK[:, cols].astype(np.float16),
            "wuv": W_UV[:, cols].astype(np.float16),
            "wout": W_out[cols, :].astype(np.float16),
            "mask": mask,
        })
    return in_maps


def run_on_device(x, W_query, W_DKV, W_UK, W_UV, W_out, **run_kwargs):
    nc = _get_nc()
    in_maps = _make_in_maps(x, W_query, W_DKV, W_UK, W_UV, W_out)
    return bass_utils.run_bass_kernel_spmd(
        nc, in_maps, core_ids=list(range(NCORES)), **run_kwargs)


def kernel(x, W_query, W_DKV, W_UK, W_UV, W_out, b_out):
    x = np.asarray(x, dtype=np.float32)
    W_query = np.asarray(W_query, dtype=np.float32)
    W_DKV = np.asarray(W_DKV, dtype=np.float32)
    W_UK = np.asarray(W_UK, dtype=np.float32)
    W_UV = np.asarray(W_UV, dtype=np.float32)
    W_out = np.asarray(W_out, dtype=np.float32)
    b_out = np.asarray(b_out, dtype=np.float32)

    res = None
    for attempt in range(3):
        try:
            res = run_on_device(x, W_query, W_DKV, W_UK, W_UV, W_out)
            break
        except Exception:
            if attempt == 2:
                raise
    out = np.empty((B, S, DOUT), dtype=np.float32)
    for b in range(B):
        acc = res.results[4 * b]["out"].astype(np.float32)
        for g in range(1, 4):
            acc += res.results[4 * b + g]["out"].astype(np.float32)
        out[b] = acc + b_out[None, :]
    return out


# revision 24
# speedup vs baseline: 1.0341x; 1.0129x over previous
"""MultiHeadLatentAttention TRN2 kernel.

Sharding: 8 cores = 2 batches x 4 head-groups (4 heads of 128 dims each).
Each core computes, for its (batch, 4 heads):
    latT = Wdkv^T xT          [256, S]
    kT_h = Wuk_h^T latT       [hd, S]
    v_h  = latT^T Wuv_h       [S, hd]   (stationary latT, amortized LDW)
    qT_h = Wq_h^T xT          [hd, S]
    scoresT = k qT            [keys, q]   (transposed scores: no transposes)
    expT = exp(scale*scoresT) (causal: skip invalid blocks, tri-mask diagonal;
                               invalid strips memset to -30000 on DVE)
    den  = ones^T quadsum(expT)  [128, q]  (4 key-tiles per den matmul)
    ctxT = v^T expT / den     [hd, q]
    part = sum_h ctxT_h^T Wout_h  [S, dout]  (row-parallel partial, fp16 out)
Host sums the 4 partials per batch and adds b_out.

Out-proj is emitted in per-stt chunks interleaved between the next
q-block's attention heads so PE fills ACT-paced bubbles.
"""

import sys

_BASS_REPO = "/opt/trn_rl_repo"
if _BASS_REPO not in sys.path:
    sys.path.insert(0, _BASS_REPO)

import numpy as np

import concourse.bass as bass  # noqa: F401
import concourse.mybir as mybir
import concourse.tile as tile
from concourse import bacc, bass_utils

F32 = mybir.dt.float32
F16 = mybir.dt.float16

B = 2
S = 2048
DIN = 2048
DOUT = 2048
NH = 16
HD = 128
LAT = 256
NCORES = 8
HEADS_PER_CORE = 4
COLS_PER_CORE = HEADS_PER_CORE * HD  # 512

KC = DIN // 128  # 16 contraction chunks over d_in
NB = S // 512    # 4 blocks of 512 over S
NT = S // 128    # 16 tiles of 128 over S
SCALE = 1.0 / float(np.sqrt(HD))

_CACHE = {}


def _build():
    nc = bacc.Bacc("TRN2", target_bir_lowering=False, debug=False,
                   num_devices=NCORES)

    xt_d = nc.dram_tensor("xt", [DIN, S], F16, kind="ExternalInput")
    wq_d = nc.dram_tensor("wq", [DIN, COLS_PER_CORE], F16, kind="ExternalInput")
    wdkv_d = nc.dram_tensor("wdkv", [DIN, LAT], F16, kind="ExternalInput")
    wuk_d = nc.dram_tensor("wuk", [LAT, COLS_PER_CORE], F16, kind="ExternalInput")
    wuv_d = nc.dram_tensor("wuv", [LAT, COLS_PER_CORE], F16, kind="ExternalInput")
    wout_d = nc.dram_tensor("wout", [COLS_PER_CORE, DOUT], F16, kind="ExternalInput")
    mask_d = nc.dram_tensor("mask", [128, 128], F16, kind="ExternalInput")
    out_d = nc.dram_tensor("out", [S, DOUT], F16, kind="ExternalOutput")

    Exp = mybir.ActivationFunctionType.Exp

    with tile.TileContext(nc) as tc:
        with (
            tc.tile_pool(name="consts", bufs=1) as cpool,
            tc.tile_pool(name="wts", bufs=1) as wpool,
            tc.tile_pool(name="acts", bufs=1) as apool,
            tc.tile_pool(name="temps", bufs=1) as tpool,
        ):
            # ---- constants ----
            ones_t = cpool.tile([128, 512], F16, name="ones_t", tag="ones_t")
            nc.vector.memset(ones_t[:], 1.0)
            neg_t = cpool.tile([128, 128], F16, name="neg_t", tag="neg_t")
            nc.vector.memset(neg_t[:], -30000.0)
            mask_t = cpool.tile([128, 128], F16, name="mask_t", tag="mask_t")
            nc.scalar.dma_start(mask_t[:], mask_d.ap())

            # ---- weights / inputs ----
            # xt streams on both rings (feeds latT then qT); wdkv rides the
            # sync ring interleaved; wuk/wuv early on scalar; wq after xt;
            # wout late on scalar.
            # wuk/wuv are tiny and needed right after latT -- load them
            # FIRST on the scalar ring so kT/v never stall on them.
            wuk = []
            wuv = []
            for m in range(LAT // 128):
                t = wpool.tile([128, COLS_PER_CORE], F16, name=f"wuk{m}", tag=f"wuk{m}")
                nc.scalar.dma_start(t[:], wuk_d.ap()[128 * m:128 * (m + 1), :])
                wuk.append(t)
                t = wpool.tile([128, COLS_PER_CORE], F16, name=f"wuv{m}", tag=f"wuv{m}")
                nc.scalar.dma_start(t[:], wuv_d.ap()[128 * m:128 * (m + 1), :])
                wuv.append(t)
            xtA = []
            xtB = []
            wdkv = []
            for k in range(KC):
                ta = wpool.tile([128, S // 2], F16, name=f"xtA{k}", tag=f"xtA{k}")
                nc.sync.dma_start(ta[:], xt_d.ap()[128 * k:128 * (k + 1), 0:S // 2])
                tb = wpool.tile([128, S // 2], F16, name=f"xtB{k}", tag=f"xtB{k}")
                nc.scalar.dma_start(tb[:], xt_d.ap()[128 * k:128 * (k + 1), S // 2:S])
                xtA.append(ta)
                xtB.append(tb)
                t = wpool.tile([128, LAT], F16, name=f"wdkv{k}", tag=f"wdkv{k}")
                eng = nc.sync if k % 2 == 0 else nc.scalar
                eng.dma_start(t[:], wdkv_d.ap()[128 * k:128 * (k + 1), :])
                wdkv.append(t)

            def xt_sb(k, sb):
                t = xtA[k] if sb < 2 else xtB[k]
                return t[:, 512 * (sb % 2):512 * (sb % 2 + 1)]
            wq = []
            for k in range(KC):
                t = wpool.tile([128, COLS_PER_CORE], F16, name=f"wq{k}", tag=f"wq{k}")
                eng = nc.scalar if k % 2 == 0 else nc.sync
                eng.dma_start(t[:], wq_d.ap()[128 * k:128 * (k + 1), :])
                wq.append(t)
            wout = []
            for h in range(HEADS_PER_CORE):
                t = wpool.tile([128, DOUT], F16, name=f"wout{h}", tag=f"wout{h}")
                nc.sync.dma_start(t[:], wout_d.ap()[128 * h:128 * (h + 1), :])
                wout.append(t)

            # ---- persistent activations ----
            latT = [apool.tile([128, S], F16, name=f"latT{m}", tag=f"latT{m}")
                    for m in range(LAT // 128)]
            qT = [apool.tile([128, S], F16, name=f"qT{h}", tag=f"qT{h}")
                  for h in range(HEADS_PER_CORE)]
            kT = [apool.tile([128, S], F16, name=f"kT{h}", tag=f"kT{h}")
                  for h in range(HEADS_PER_CORE)]
            # vt_all column layout: [stt, h, d] -> col (4*stt + h)*128 + d
            vt_all = apool.tile([128, NT * 512], F16, name="vt_all", tag="vt_all")
            ctxT = [apool.tile([128, S], F16, name=f"ctxT{h}", tag=f"ctxT{h}")
                    for h in range(HEADS_PER_CORE)]

            # ================= phase 1: projections =================
            with tc.tile_pool(name="pproj", bufs=8, space="PSUM") as pproj:
                # HAM warm-keeper: stream dummy matmuls across the DMA ramp
                # so the PE is warm when the first real matmul issues.
                warm = pproj.tile([128, 512], F32, name="warm", tag="pp")
                for _ in range(72):
                    nc.tensor.matmul(warm[:, 0:128], ones_t[:, 0:128], ones_t[:, 0:128],
                                     start=True, stop=True)

                def kmajor(groups, lhs_of, rhs_of, nk, out_of, copy_eng,
                           filler=0):
                    """Accumulate len(groups) psum banks over nk chunks,
                    chunk-major so compute starts on the first DMA. filler>0
                    emits dummy ldweights between chunk iterations: they keep
                    the HAM clock-gate warm through DMA-paced stretches and
                    are harmless (every matmul reloads its own stationary)."""
                    pls = [pproj.tile([128, 512], F32, name=f"pp{i}", tag="pp")
                           for i in range(len(groups))]
                    for k in range(nk):
                        for i, g in enumerate(groups):
                            nc.tensor.matmul(pls[i][:], lhs_of(k, g), rhs_of(k, g),
                                             start=(k == 0), stop=(k == nk - 1))
                        if k < nk - 1:
                            for _ in range(filler):
                                nc.tensor.ldweights(ones_t[:, 0:128])
                    for i, g in enumerate(groups):
                        eng = copy_eng(i)
                        if eng == "scalar":
                            nc.scalar.copy(out_of(g), pls[i][:])
                        else:
                            nc.vector.tensor_copy(out_of(g), pls[i][:])

                # latT = Wdkv^T xT   (8 groups: 2 m x 4 sb)
                kmajor(
                    [(m, sb) for m in range(2) for sb in range(NB)],
                    lambda k, g: wdkv[k][:, 128 * g[0]:128 * (g[0] + 1)],
                    lambda k, g: xt_sb(k, g[1]),
                    KC,
                    lambda g: latT[g[0]][:, 512 * g[1]:512 * (g[1] + 1)],
                    lambda i: "vector" if i % 2 else "scalar", filler=6)

                # kT_h = Wuk_h^T latT  (two rounds of 2h x 4sb, 2 chunks)
                for h0 in (0, 2):
                    kmajor(
                        [(h0 + dh, sb) for dh in range(2) for sb in range(NB)],
                        lambda k, g: wuk[k][:, 128 * g[0]:128 * (g[0] + 1)],
                        lambda k, g: latT[k][:, 512 * g[1]:512 * (g[1] + 1)],
                        2,
                        lambda g: kT[g[0]][:, 512 * g[1]:512 * (g[1] + 1)],
                        lambda i: "vector" if i % 2 else "scalar")

                # v_h = latT^T Wuv_h: stationary latT s-tile reused across
                # the 4 heads (LDW amortized). Each m-chunk goes to its own
                # bank as independent start/stop groups (interleaved groups
                # on one bank would clear each other's has_written bits);
                # the two banks are summed on the way to SBUF.
                for stt in range(NT):
                    pv0 = pproj.tile([128, 512], F32, name="pv0", tag="pp")
                    pv1 = pproj.tile([128, 512], F32, name="pv1", tag="pp")
                    for m, pv in ((0, pv0), (1, pv1)):
                        for h in range(HEADS_PER_CORE):
                            nc.tensor.matmul(
                                pv[:, 128 * h:128 * (h + 1)],
                                latT[m][:, 128 * stt:128 * (stt + 1)],
                                wuv[m][:, 128 * h:128 * (h + 1)],
                                start=True, stop=True,
                                skip_group_check=True)
                    vtmp = tpool.tile([128, 512], F16, name="vtmp",
                                      tag="vtmp", bufs=2)
                    nc.scalar.copy(vtmp[:], pv1[:])
                    nc.vector.tensor_add(
                        vt_all[:, 512 * stt:512 * (stt + 1)], pv0[:], vtmp[:])

                # qT_h = Wq_h^T xT (last: its tail is the phase barrier;
                # copies split across both engines to drain fast)
                for h0 in (0, 2):
                    kmajor(
                        [(h0 + dh, sb) for dh in range(2) for sb in range(NB)],
                        lambda k, g: wq[k][:, 128 * g[0]:128 * (g[0] + 1)],
                        lambda k, g: xt_sb(k, g[1]),
                        KC,
                        lambda g: qT[g[0]][:, 512 * g[1]:512 * (g[1] + 1)],
                        lambda i: "vector" if i % 2 else "scalar")

            # ========= phase 2: attention + interleaved out-proj =========
            with (
                tc.tile_pool(name="psc", bufs=2, space="PSUM") as psc,
                tc.tile_pool(name="pctx", bufs=2, space="PSUM") as pctx,
                tc.tile_pool(name="pden", bufs=2, space="PSUM") as pden,
            ):
                def vslice(kt, h):
                    c0 = (4 * kt + h) * 128
                    return vt_all[:, c0:c0 + 128]

                def attention(qb, h):
                    ps_ctx = pctx.tile([128, 512], F32, name="ps_ctx", tag="ctx")
                    ps_den = pden.tile([128, 512], F32, name="ps_den", tag="den")
                    nkt = 4 * qb + 4
                    npair = nkt // 2
                    state = {}

                    def part1(p):
                        # scores + exp + mask for pair p
                        kt0 = 2 * p
                        pair = (kt0, kt0 + 1)
                        djA = pair[0] - 4 * qb
                        col0 = 128 * djA if djA >= 0 else 0
                        qhi = 512 * (qb + 1)
                        ps_sc = psc.tile([128, 1024], F32, name="ps_sc",
                                         tag="sc")
                        ex = tpool.tile([128, 1024], F16, name="ex", tag="ex",
                                        bufs=3)
                        # invalid strip of half 1 (valid starts 128 later
                        # than half 0): memset -huge on DVE so the wide exp
                        # lands exact zeros there (was a PE fill matmul).
                        dj1 = pair[1] - 4 * qb
                        if dj1 > 0:
                            nc.vector.memset(
                                ps_sc[:, 512 + col0:512 + 128 * dj1],
                                -30000.0)
                        for half, kt in enumerate(pair):
                            dj = kt - 4 * qb
                            c = 128 * dj if dj >= 0 else 0
                            nc.tensor.matmul(
                                ps_sc[:, 512 * half + c:512 * (half + 1)],
                                kT[h][:, 128 * kt:128 * (kt + 1)],
                                qT[h][:, 512 * qb + c:qhi],
                                start=True, stop=True,
                                skip_group_check=True)
                        # one wide exp for the pair (psum -> sbuf fp16)
                        nc.scalar.activation(ex[:, col0:1024],
                                             ps_sc[:, col0:1024], Exp,
                                             scale=SCALE)
                        for half, kt in enumerate(pair):
                            dj = kt - 4 * qb
                            if dj >= 0:
                                c = 128 * dj
                                sl = ex[:, 512 * half + c:512 * half + c + 128]
                                nc.vector.tensor_mul(sl, sl, mask_t[:])
                        state[p] = (ex, col0)

                    def part2(p):
                        # ctx + den matmuls for pair p
                        kt0 = 2 * p
                        pair = (kt0, kt0 + 1)
                        ex, col0 = state.pop(p)
                        for half, kt in enumerate(pair):
                            nc.tensor.matmul(
                                ps_ctx[:, col0:512],
                                vslice(kt, h),
                                ex[:, 512 * half + col0:512 * (half + 1)],
                                start=(kt0 == 0 and half == 0),
                                stop=(kt == nkt - 1))
                        # pair-sum on DVE; den matmul once per quad
                        exs = tpool.tile([128, 512], F16, name="exs",
                                         tag="exs", bufs=3)
                        if col0 > 0:
                            nc.vector.memset(exs[:, 0:col0], 0.0)
                        nc.vector.tensor_add(exs[:, col0:512],
                                             ex[:, col0:512],
                                             ex[:, 512 + col0:1024])
                        if p % 2 == 0:
                            state['exs'] = exs
                        else:
                            quad = tpool.tile([128, 512], F16, name="quad",
                                              tag="quad", bufs=3)
                            nc.vector.tensor_add(quad[:], state.pop('exs'),
                                                 exs[:])
                            nc.tensor.matmul(
                                ps_den[:], ones_t[:, 0:128], quad[:],
                                start=(p == 1), stop=(p == npair - 1))

                    # one-deep software pipeline: pair p+1's scores issue
                    # between pair p's scores and its ctx/den matmuls, so
                    # the PE fills the exp/mask latency.
                    part1(0)
                    for p in range(1, npair):
                        part1(p)
                        part2(p - 1)
                    part2(npair - 1)
                    rden = tpool.tile([128, 512], F32, name="rden", tag="rden",
                                      bufs=3)
                    nc.vector.reciprocal_approx_fast(rden[:], ps_den[:])
                    nc.vector.tensor_mul(ctxT[h][:, 512 * qb:512 * (qb + 1)],
                                         ps_ctx[:], rden[:])

                def outproj_chunk(stt, tail):
                    for ob in range(NB):
                        po = pden.tile([128, 512], F32, name="po", tag="den")
                        for h in range(HEADS_PER_CORE):
                            nc.tensor.matmul(
                                po[:],
                                ctxT[h][:, 128 * stt:128 * (stt + 1)],
                                wout[h][:, 512 * ob:512 * (ob + 1)],
                                start=(h == 0), stop=(h == HEADS_PER_CORE - 1))
                        osb = tpool.tile([128, 512], F16, name="osb",
                                         tag="osb", bufs=4)
                        if ob % 2 == 0:
                            nc.scalar.copy(osb[:], po[:])
                        else:
                            nc.vector.tensor_copy(osb[:], po[:])
                        st_eng = nc.sync if ob % 2 == 0 else nc.scalar
                        st_eng.dma_start(
                            out_d.ap()[128 * stt:128 * (stt + 1),
                                       512 * ob:512 * (ob + 1)],
                            osb[:])

                # out-proj chunks of q-block qb-1 interleave between the
                # attention heads of q-block qb (fills ACT-paced bubbles)
                # ldweights bridge across the pool-swap barrier (no PSUM
                # dependency, so they run while the projection pools drain)
                for _ in range(16):
                    nc.tensor.ldweights(ones_t[:, 0:128])
                for qb in range(NB):
                    for h in range(HEADS_PER_CORE):
                        attention(qb, h)
                        if qb <= 1:
                            for _ in range(8):
                                nc.tensor.ldweights(ones_t[:, 0:128])
                        if qb > 0:
                            outproj_chunk(4 * (qb - 1) + h, tail=False)
                for stt in range(12, 16):
                    outproj_chunk(stt, tail=True)

    nc.compile()
    return nc


def _get_nc():
    if "nc" not in _CACHE:
        _CACHE["nc"] = _build()
    return _CACHE["nc"]


def _make_in_maps(x, W_query, W_DKV, W_UK, W_UV, W_out):
    mask = np.triu(np.ones((128, 128), dtype=np.float16))
    wdkv16 = W_DKV.astype(np.float16)
    xT16 = [x[b].T.astype(np.float16) for b in range(B)]
    in_maps = []
    for c in range(NCORES):
        b = c // 4
        g = c % 4
        cols = slice(512 * g, 512 * (g + 1))
        in_maps.append({
            "xt": xT16[b],
            "wq": W_query[:, cols].astype(np.float16),
            "wdkv": wdkv16,
            "wuk": W_UK[:, cols].astype(np.float16),
            "wuv": W_UV[:, cols].astype(np.float16),
            "wout": W_out[cols, :].astype(np.float16),
            "mask": mask,
        })
    return in_maps


def run_on_device(x, W_query, W_DKV, W_UK, W_UV, W_out, **run_kwargs):
    nc = _get_nc()
    in_maps = _make_in_maps(x, W_query, W_DKV, W_UK, W_UV, W_out)
    return bass_utils.run_bass_kernel_spmd(
        nc, in_maps, core_ids=list(range(NCORES)), **run_kwargs)


def kernel(x, W_query, W_DKV, W_UK, W_UV, W_out, b_out):
    x = np.asarray(x, dtype=np.float32)
    W_query = np.asarray(W_query, dtype=np.float32)
    W_DKV = np.asarray(W_DKV, dtype=np.float32)
    W_UK = np.asarray(W_UK, dtype=np.float32)
    W_UV = np.asarray(W_UV, dtype=np.float32)
    W_out = np.asarray(W_out, dtype=np.float32)
    b_out = np.asarray(b_out, dtype=np.float32)

    res = None
    for attempt in range(3):
        try:
            res = run_on_device(x, W_query, W_DKV, W_UK, W_UV, W_out)
            break
        except Exception:
            if attempt == 2:
                raise
    out = np.empty((B, S, DOUT), dtype=np.float32)
    for b in range(B):
        acc = res.results[4 * b]["out"].astype(np.float32)
        for g in range(1, 4):
            acc += res.results[4 * b + g]["out"].astype(np.float32)
        out[b] = acc + b_out[None, :]
    return out
